# revision 16
# baseline (speedup 1.0000x reference)
"""Single-head causal attention (B=8, T=2048, C=1024, H=64) on 8 NeuronCores.

Data-parallel over batch: core b computes attention for x[b].

v8 design notes (v1 122.6us ... v6 72.2us):
  * Host stages x transposed; weights host-packed; 1/sqrt(H) folded
    into Wq; bf16 identity shipped from host.
  * Projections are three M=64 col-tiled streams per chunk (k -> PSUM
    partitions 0-63, q -> 64-127 of the same bank, v -> a second bank):
    adjacent matmuls on disjoint column strips run concurrently in the
    PE array, and q lands directly in the partition-64 copy the odd
    score matmuls need (kT-hi / qT-lo copies via SBUF->SBUF DMA).
  * PV drops the ones-column (M=64) and the softmax row sums come from
    a concurrent M=1 col-tiled matmul (stationary ones vector) into
    partition 64 of the same accumulator bank, so a PV step costs one
    N=512 stream instead of a serial M=65 matmul.
  * Attention is one global software pipeline: S-pairs (row-packed 2x
    via partition-64 operand copies) run ahead, exp+mask chase, PV
    chases; projection/v-transpose fillers and low-priority dummy
    matmuls keep the PE dense so HAM stays at K=8/8.
  * Diagonal tiles use reduced query width for S/exp/mask/PV/sums.
"""

import numpy as np
import ml_dtypes

import concourse.bass as bass
import concourse.bacc as bacc
import concourse.mybir as mybir
import concourse.tile as tile
from concourse.bass_utils import run_bass_kernel_spmd

B = 8
T, C, H = 2048, 1024, 64
P = 128
NCH = C // P     # 8 C-chunks
NT = T // P      # 16 T-tiles
QT = 512         # query-block width
NQ = T // QT     # 4 query blocks
H1 = H + 1
f32 = mybir.dt.float32
bf16 = mybir.dt.bfloat16
EXP = mybir.ActivationFunctionType.Exp
BF16NP = np.dtype(ml_dtypes.bfloat16)


def build_nc() -> bass.Bass:
    nc = bacc.Bacc("TRN2", target_bir_lowering=False, debug=False)
    xT = nc.dram_tensor("xT", [C, T], f32, kind="ExternalInput")
    Wkq = nc.dram_tensor("Wkq", [P, NCH * P], f32, kind="ExternalInput")
    Wvp = nc.dram_tensor("Wvp", [P, NCH * H], f32, kind="ExternalInput")
    IdD = nc.dram_tensor("IdD", [P, P], bf16, kind="ExternalInput")
    out = nc.dram_tensor("out", [T, H], f32, kind="ExternalOutput")

    with tile.TileContext(nc) as tc:
        with (
            tc.tile_pool(name="const", bufs=1) as constp,
            tc.tile_pool(name="w", bufs=1) as wp,
            tc.tile_pool(name="xt", bufs=3) as xtp,
            tc.tile_pool(name="qkv", bufs=1) as qkvp,
            tc.tile_pool(name="pt", bufs=6) as ptp,
            tc.tile_pool(name="fin", bufs=4) as finp,
            tc.tile_pool(name="qk", bufs=1, space="PSUM") as qkp,    # k|q chains
            tc.tile_pool(name="vv", bufs=1, space="PSUM") as vvp,    # v chains
            tc.tile_pool(name="sps", bufs=4, space="PSUM") as spsp,  # S/pv/pob
            tc.tile_pool(name="acc", bufs=1, space="PSUM") as accp,  # po
            tc.tile_pool(name="junk", bufs=1, space="PSUM") as junkp,
        ):
            # identity from host via sync HWDGE (nothing queued ahead of it)
            ident = constp.tile([P, P], bf16, tag="ident")
            nc.sync.dma_start(out=ident, in_=IdD[:, :])

            # --- gpsimd SWDGE queue: weights, then per-chunk x windows ---
            wkq_r = wp.tile([P, NCH * P], bf16, tag="wkq_r")
            wv_r = wp.tile([P, NCH * H], bf16, tag="wv_r")
            nc.gpsimd.dma_start(out=wkq_r, in_=Wkq[:, :])
            nc.gpsimd.dma_start(out=wv_r, in_=Wvp[:, :])
            xvs = []
            for w in range(NQ):
                xtw = xtp.tile([P, NCH * QT], bf16, tag="xtw", name=f"xtw{w}")
                xv = xtw.rearrange("p (c t) -> p c t", t=QT)
                for c in range(NCH):
                    nc.gpsimd.dma_start(
                        out=xv[:, c, :],
                        in_=xT[c * P : (c + 1) * P, w * QT : (w + 1) * QT])
                xvs.append(xv)

            # --- persistent SBUF tensors ---
            kq = qkvp.tile([P, 2 * T], bf16, tag="kq")   # [0:T]=kT, [T:2T]=qT
            vt = qkvp.tile([P, T], bf16, tag="vt")       # vT at partitions 64-127
            vsb = qkvp.tile([P, NT * H], bf16, tag="vsb")  # v natural
            vsb_v = vsb.rearrange("p (t w) -> p t w", w=H)
            ones = constp.tile([P, 1], bf16, tag="ones")
            nc.vector.memset(ones, 1.0)
            osb = finp.tile([P, NT * H], f32, tag="osb", bufs=1)

            # --- reusable dummy-matmul filler (keeps HAM busy), low prio ---
            jt = junkp.tile([P, P], f32, tag="junk")

            def dummy_mm():
                old = tc.cur_priority
                tc.cur_priority = old + 100000
                nc.tensor.matmul(jt, ident, ident, start=True, stop=True)
                tc.cur_priority = old

            def project_fillers(w):
                """PE-op closures for projections + v-transpose of window w.

                Three col-tiled M=64 streams per chunk: k -> qk[0:64],
                q -> qk[64:128] (same bank, concurrent strips), v -> a
                second bank's partitions 64-127."""
                xv = xvs[w]
                qkps = qkp.tile([P, QT], f32, tag="qk", name=f"qk{w}")
                vps = vvp.tile([P, QT], f32, tag="vv", name=f"v{w}")
                ops = []
                for c in range(NCH):
                    ops.append(lambda c=c: nc.tensor.matmul(
                        qkps[0:H, :], wkq_r[:, c * P : c * P + H], xv[:, c, :],
                        start=(c == 0), stop=(c == NCH - 1)))
                    ops.append(lambda c=c: nc.tensor.matmul(
                        qkps[H:P, :], wkq_r[:, c * P + H : (c + 1) * P],
                        xv[:, c, :],
                        start=(c == 0), stop=(c == NCH - 1)))
                    ops.append(lambda c=c: nc.tensor.matmul(
                        vps[H:P, :], wv_r[:, c * H : (c + 1) * H], xv[:, c, :],
                        start=(c == 0), stop=(c == NCH - 1)))

                def casts():
                    cols = slice(w * QT, (w + 1) * QT)
                    qcols = slice(T + w * QT, T + (w + 1) * QT)
                    with tc.high_priority():
                        nc.vector.tensor_copy(kq[0:H, cols], qkps[0:H, :])
                        nc.vector.tensor_copy(kq[H:P, qcols], qkps[H:P, :])
                        nc.vector.tensor_copy(vt[H:P, cols], vps[H:P, :])
                        # duplicate kT lo->hi and qT hi->lo
                        nc.sync.dma_start(out=kq[H:P, cols], in_=kq[0:H, cols])
                        nc.sync.dma_start(out=kq[0:H, qcols], in_=kq[H:P, qcols])
                ops.append(casts)

                pv = spsp.tile([P, 4 * H], f32, tag="sps", name=f"pv{w}")
                for k in range(4):
                    ops.append(lambda k=k: nc.tensor.matmul(
                        pv[:, k * H : (k + 1) * H],
                        vt[H:P, (4 * w + k) * P : (4 * w + k + 1) * P],
                        ident[H:P, H:P], start=True, stop=True))
                ops.append(lambda: nc.vector.tensor_copy(
                    vsb_v[:, 4 * w : 4 * w + 4, :],
                    pv.rearrange("p (t u) -> p t u", u=H)))
                return ops

            # ---- global attention pipeline across blocks ----
            def width(i, j):
                d = j - 4 * i
                return QT - d * P if d > 0 else QT

            def s_mm(i, j):
                w = width(i, j)
                ps = spsp.tile([P, QT], f32, tag="sps", name=f"s{i}_{j}")
                rows = slice(0, H) if j % 2 == 0 else slice(H, P)
                qoff = T + i * QT + (QT - w)
                with tc.high_priority():
                    nc.tensor.matmul(
                        ps[:, 0:w],
                        kq[rows, j * P : (j + 1) * P],
                        kq[rows, qoff : qoff + w],
                        start=True, stop=True)
                return ps

            def exp_mask(i, j, ps):
                w = width(i, j)
                pt = ptp.tile([P, QT], bf16, tag="pt", name=f"pt{i}_{j}")
                with tc.high_priority():
                    nc.scalar.activation(pt[:, 0:w], ps[:, 0:w], EXP)
                    if j >= 4 * i:
                        nc.gpsimd.affine_select(
                            out=pt[:, 0:w], in_=pt[:, 0:w],
                            pattern=[[1, w]],
                            compare_op=mybir.AluOpType.is_ge, fill=0.0,
                            base=0, channel_multiplier=-1)
                return pt

            def finish_block(i, po):
                ot = finp.tile([H1, QT], bf16, tag="ot")
                nc.vector.tensor_copy(ot, po[0:H1, :])
                pob = spsp.tile([P, 4 * H1], f32, tag="sps", name=f"pob{i}")
                for b in range(4):
                    nc.tensor.matmul(
                        pob[:, b * H1 : (b + 1) * H1],
                        ot[:, b * P : (b + 1) * P],
                        ident[0:H1, 0:H1], start=True, stop=True)
                for b in range(4):
                    t = 4 * i + b
                    rcp = finp.tile([P, 1], f32, tag="rcp")
                    nc.vector.reciprocal(rcp, pob[:, b * H1 + H : b * H1 + H1])
                    nc.vector.tensor_scalar_mul(
                        osb[:, t * H : (t + 1) * H],
                        pob[:, b * H1 : b * H1 + H], rcp)
                nc.sync.dma_start(
                    out=out.rearrange("(t p) h -> p t h", p=P)[:, 4 * i : 4 * i + 4, :],
                    in_=osb.rearrange("p (t h) -> p t h", h=H)[:, 4 * i : 4 * i + 4, :])

            steps = [(i, k) for i in range(NQ) for k in range(2 * (i + 1))]
            nsteps = len(steps)

            # windows 0 and 1 projected up front (DMA-paced anyway);
            # window w+2 projected as fillers inside attention block w
            for op in project_fillers(0):
                op()
            if NQ > 1:
                for op in project_fillers(1):
                    op()

            state = {"fillers": [], "fillers_w": 1, "proj_emitted": 1,
                     "s_ptr": 0}
            if NQ > 2:
                state["fillers"] = project_fillers(2)
                state["fillers_w"] = 2
            POPS = {0: 8, 1: 6, 2: 5, 3: 6}
            pss = {}
            pos = {}

            def pop_filler():
                if state["fillers"]:
                    state["fillers"].pop(0)()
                    if not state["fillers"]:
                        state["proj_emitted"] = max(
                            state["proj_emitted"], state["fillers_w"])
                else:
                    dummy_mm()

            def drain_fillers():
                while state["fillers"]:
                    state["fillers"].pop(0)()
                state["proj_emitted"] = max(
                    state["proj_emitted"], state["fillers_w"])

            def emit_S_upto(limit):
                while state["s_ptr"] < min(limit, nsteps):
                    si, sk = steps[state["s_ptr"]]
                    if si > state["proj_emitted"]:
                        break
                    for j in (2 * sk, 2 * sk + 1):
                        pss[(si, j)] = s_mm(si, j)
                    state["s_ptr"] += 1

            cur_block = 0
            emit_S_upto(2)
            for s, (i, k) in enumerate(steps):
                if i != cur_block:
                    drain_fillers()
                    cur_block = i
                    if i + 2 < NQ:
                        state["fillers"] = project_fillers(i + 2)
                        state["fillers_w"] = i + 2
                    emit_S_upto(s + 2)
                nj = 4 * (i + 1)
                if k == 0:
                    pos[i] = accp.tile([P, QT], f32, tag="po", name=f"po{i}")
                po = pos[i]
                pts = {}
                for j in (2 * k, 2 * k + 1):
                    pts[j] = exp_mask(i, j, pss.pop((i, j)))
                emit_S_upto(s + 3)
                for j in (2 * k, 2 * k + 1):
                    w = width(i, j)
                    pt = pts.pop(j)
                    # PV (M=64, strips 0-1) + row sums (M=1, strip 2,
                    # partition 64 of the same bank) run concurrently
                    nc.tensor.matmul(
                        po[0:H, QT - w : QT],
                        vsb[:, j * H : (j + 1) * H],
                        pt[:, 0:w],
                        start=(j == 0), stop=(j == nj - 1))
                    nc.tensor.matmul(
                        po[H : H + 1, QT - w : QT],
                        ones,
                        pt[:, 0:w],
                        start=(j == 0), stop=(j == nj - 1))
                for _ in range(POPS.get(i, 3)):
                    pop_filler()
                if k == 2 * (i + 1) - 1:
                    finish_block(i, pos.pop(i))

    nc.compile()
    return nc


_NC_CACHE = None


def _get_nc():
    global _NC_CACHE
    if _NC_CACHE is None:
        _NC_CACHE = build_nc()
    return _NC_CACHE


def run(in_maps, trace=False, **kw):
    nc = _get_nc()
    return run_bass_kernel_spmd(nc, in_maps, core_ids=list(range(B)),
                                trace=trace, **kw)


def _pack_weights(Wq, Wk, Wv):
    """Host-side layout packing (pure permutation + constant folding)."""
    wkq = np.empty((P, NCH * P), dtype=np.float32)
    wv = np.empty((P, NCH * H), dtype=np.float32)
    scale = np.float32(1.0 / np.sqrt(H))
    for c in range(NCH):
        rows = slice(c * P, (c + 1) * P)
        wkq[:, c * P : c * P + H] = Wk[rows, :]
        wkq[:, c * P + H : (c + 1) * P] = Wq[rows, :] * scale
        wv[:, c * H : (c + 1) * H] = Wv[rows, :]
    return wkq, wv


def make_in_maps(x, Wq, Wk, Wv):
    x = np.asarray(x, dtype=np.float32)
    Wq = np.asarray(Wq, dtype=np.float32)
    Wk = np.asarray(Wk, dtype=np.float32)
    Wv = np.asarray(Wv, dtype=np.float32)
    wkq, wv = _pack_weights(Wq, Wk, Wv)
    ident = np.eye(P, dtype=BF16NP)
    return [
        {"xT": np.ascontiguousarray(x[b].T), "Wkq": wkq, "Wvp": wv,
         "IdD": ident}
        for b in range(B)
    ]


def kernel(x, Wq, Wk, Wv):
    res = run(make_in_maps(x, Wq, Wk, Wv))
    return np.stack([res.results[b]["out"] for b in range(B)], axis=0)


# revision 19
# speedup vs baseline: 1.1662x; 1.1662x over previous
"""Single-head causal attention (B=8, T=2048, C=1024, H=64) on 8 NeuronCores.

Data-parallel over batch: core b computes attention for x[b].

v8 design notes (v1 122.6us ... v6 72.2us):
  * Host stages x transposed; weights host-packed; 1/sqrt(H) folded
    into Wq; bf16 identity shipped from host.
  * Projections are three M=64 col-tiled streams per chunk (k -> PSUM
    partitions 0-63, q -> 64-127 of the same bank, v -> a second bank):
    adjacent matmuls on disjoint column strips run concurrently in the
    PE array, and q lands directly in the partition-64 copy the odd
    score matmuls need (kT-hi / qT-lo copies via SBUF->SBUF DMA).
  * PV drops the ones-column (M=64) and the softmax row sums come from
    a concurrent M=1 col-tiled matmul (stationary ones vector) into
    partition 64 of the same accumulator bank, so a PV step costs one
    N=512 stream instead of a serial M=65 matmul.
  * Attention is one global software pipeline: S-pairs (row-packed 2x
    via partition-64 operand copies) run ahead, exp+mask chase, PV
    chases; projection/v-transpose fillers and low-priority dummy
    matmuls keep the PE dense so HAM stays at K=8/8.
  * Diagonal tiles use reduced query width for S/exp/mask/PV/sums.
"""

import numpy as np
import ml_dtypes

import concourse.bass as bass
import concourse.bacc as bacc
import concourse.mybir as mybir
import concourse.tile as tile
from concourse.bass_utils import run_bass_kernel_spmd

B = 8
T, C, H = 2048, 1024, 64
P = 128
NCH = C // P     # 8 C-chunks
NT = T // P      # 16 T-tiles
QT = 512         # query-block width
NQ = T // QT     # 4 query blocks
H1 = H + 1
f32 = mybir.dt.float32
bf16 = mybir.dt.bfloat16
EXP = mybir.ActivationFunctionType.Exp
BF16NP = np.dtype(ml_dtypes.bfloat16)


def build_nc() -> bass.Bass:
    nc = bacc.Bacc("TRN2", target_bir_lowering=False, debug=False)
    xT = nc.dram_tensor("xT", [C, T], f32, kind="ExternalInput")
    Wkq = nc.dram_tensor("Wkq", [P, NCH * P], f32, kind="ExternalInput")
    Wvp = nc.dram_tensor("Wvp", [P, NCH * H], f32, kind="ExternalInput")
    IdD = nc.dram_tensor("IdD", [P, P], bf16, kind="ExternalInput")
    out = nc.dram_tensor("out", [T, H], f32, kind="ExternalOutput")

    with tile.TileContext(nc) as tc:
        with (
            tc.tile_pool(name="const", bufs=1) as constp,
            tc.tile_pool(name="w", bufs=1) as wp,
            tc.tile_pool(name="xt", bufs=3) as xtp,
            tc.tile_pool(name="qkv", bufs=1) as qkvp,
            tc.tile_pool(name="pt", bufs=6) as ptp,
            tc.tile_pool(name="fin", bufs=4) as finp,
            tc.tile_pool(name="ps", bufs=2, space="PSUM") as psp,    # kv/q chains
            tc.tile_pool(name="sps", bufs=4, space="PSUM") as spsp,  # S/pv/pob
            tc.tile_pool(name="acc", bufs=1, space="PSUM") as accp,  # po
            tc.tile_pool(name="junk", bufs=1, space="PSUM") as junkp,
        ):
            # identity from host via sync HWDGE (nothing queued ahead of it)
            ident = constp.tile([P, P], bf16, tag="ident")
            nc.sync.dma_start(out=ident, in_=IdD[:, :])

            # --- gpsimd SWDGE queue: weights, then per-chunk x windows ---
            wkq_r = wp.tile([P, NCH * P], bf16, tag="wkq_r")
            wv_r = wp.tile([P, NCH * H], bf16, tag="wv_r")
            nc.gpsimd.dma_start(out=wkq_r, in_=Wkq[:, :])
            nc.gpsimd.dma_start(out=wv_r, in_=Wvp[:, :])
            xvs = []
            for w in range(NQ):
                xtw = xtp.tile([P, NCH * QT], bf16, tag="xtw", name=f"xtw{w}")
                xv = xtw.rearrange("p (c t) -> p c t", t=QT)
                for c in range(NCH):
                    nc.gpsimd.dma_start(
                        out=xv[:, c, :],
                        in_=xT[c * P : (c + 1) * P, w * QT : (w + 1) * QT])
                xvs.append(xv)

            # --- persistent SBUF tensors ---
            kq = qkvp.tile([P, 2 * T], bf16, tag="kq")   # [0:T]=kT, [T:2T]=qT
            vt = qkvp.tile([P, T], bf16, tag="vt")       # vT at partitions 64-127
            vsb = qkvp.tile([P, NT * H], bf16, tag="vsb")  # v natural
            vsb_v = vsb.rearrange("p (t w) -> p t w", w=H)
            ones = constp.tile([P, 1], bf16, tag="ones")
            nc.vector.memset(ones, 1.0)
            osb = finp.tile([P, NT * H], f32, tag="osb", bufs=1)

            # --- reusable dummy-matmul filler (keeps HAM busy), low prio ---
            jt = junkp.tile([P, P], f32, tag="junk")

            def dummy_mm():
                old = tc.cur_priority
                tc.cur_priority = old + 100000
                nc.tensor.matmul(jt, ident, ident, start=True, stop=True)
                tc.cur_priority = old

            def project_fillers(w):
                """PE-op closures for projections + v-transpose of window w.

                kv packed (M=128) + q (M=64) per chunk, interleaved so the
                chain advances at DMA chunk-arrival pace."""
                xv = xvs[w]
                kvp = psp.tile([P, QT], f32, tag="big", name=f"kv{w}")
                qp = psp.tile([P, QT], f32, tag="big", name=f"q{w}")
                ops = []
                for c in range(NCH):
                    ops.append(lambda c=c: nc.tensor.matmul(
                        kvp, wkq_r[:, c * P : (c + 1) * P], xv[:, c, :],
                        start=(c == 0), stop=(c == NCH - 1)))
                    ops.append(lambda c=c: nc.tensor.matmul(
                        qp[0:H, :], wv_r[:, c * H : (c + 1) * H], xv[:, c, :],
                        start=(c == 0), stop=(c == NCH - 1)))

                def casts():
                    cols = slice(w * QT, (w + 1) * QT)
                    qcols = slice(T + w * QT, T + (w + 1) * QT)
                    with tc.high_priority():
                        nc.vector.tensor_copy(kq[0:H, cols], kvp[0:H, :])
                        nc.vector.tensor_copy(kq[0:H, qcols], qp[0:H, :])
                        nc.vector.tensor_copy(vt[H:P, cols], kvp[H:P, :])
                        kq_pair = kq.rearrange("p (s t) -> p s t", s=2)
                        nc.sync.dma_start(
                            out=kq_pair[H:P, :, w * QT : (w + 1) * QT],
                            in_=kq_pair[0:H, :, w * QT : (w + 1) * QT])
                ops.append(casts)

                pv = spsp.tile([P, 4 * H], f32, tag="sps", name=f"pv{w}")
                for k in range(4):
                    ops.append(lambda k=k: nc.tensor.matmul(
                        pv[:, k * H : (k + 1) * H],
                        vt[H:P, (4 * w + k) * P : (4 * w + k + 1) * P],
                        ident[H:P, H:P], start=True, stop=True))
                ops.append(lambda: nc.vector.tensor_copy(
                    vsb_v[:, 4 * w : 4 * w + 4, :],
                    pv.rearrange("p (t u) -> p t u", u=H)))
                return ops

            # ---- global attention pipeline across blocks ----
            def width(i, j):
                d = j - 4 * i
                return QT - d * P if d > 0 else QT

            def s_mm(i, j):
                w = width(i, j)
                ps = spsp.tile([P, QT], f32, tag="sps", name=f"s{i}_{j}")
                rows = slice(0, H) if j % 2 == 0 else slice(H, P)
                qoff = T + i * QT + (QT - w)
                with tc.high_priority():
                    nc.tensor.matmul(
                        ps[:, 0:w],
                        kq[rows, j * P : (j + 1) * P],
                        kq[rows, qoff : qoff + w],
                        start=True, stop=True)
                return ps

            def exp_mask(i, j, ps):
                w = width(i, j)
                pt = ptp.tile([P, QT], bf16, tag="pt", name=f"pt{i}_{j}")
                with tc.high_priority():
                    nc.scalar.activation(pt[:, 0:w], ps[:, 0:w], EXP)
                    if j >= 4 * i:
                        nc.gpsimd.affine_select(
                            out=pt[:, 0:w], in_=pt[:, 0:w],
                            pattern=[[1, w]],
                            compare_op=mybir.AluOpType.is_ge, fill=0.0,
                            base=0, channel_multiplier=-1)
                return pt

            def finish_block(i, po):
                ot = finp.tile([H1, QT], bf16, tag="ot")
                nc.vector.tensor_copy(ot, po[0:H1, :])
                pob = spsp.tile([P, 4 * H1], f32, tag="sps", name=f"pob{i}")
                for b in range(4):
                    nc.tensor.matmul(
                        pob[:, b * H1 : (b + 1) * H1],
                        ot[:, b * P : (b + 1) * P],
                        ident[0:H1, 0:H1], start=True, stop=True)
                for b in range(4):
                    t = 4 * i + b
                    rcp = finp.tile([P, 1], f32, tag="rcp")
                    nc.vector.reciprocal(rcp, pob[:, b * H1 + H : b * H1 + H1])
                    nc.vector.tensor_scalar_mul(
                        osb[:, t * H : (t + 1) * H],
                        pob[:, b * H1 : b * H1 + H], rcp)
                nc.sync.dma_start(
                    out=out.rearrange("(t p) h -> p t h", p=P)[:, 4 * i : 4 * i + 4, :],
                    in_=osb.rearrange("p (t h) -> p t h", h=H)[:, 4 * i : 4 * i + 4, :])

            steps = [(i, k) for i in range(NQ) for k in range(2 * (i + 1))]
            nsteps = len(steps)

            # windows 0 and 1 projected up front (DMA-paced anyway);
            # window w+2 projected as fillers inside attention block w
            for op in project_fillers(0):
                op()
            if NQ > 1:
                for op in project_fillers(1):
                    op()

            state = {"fillers": [], "fillers_w": 1, "proj_emitted": 1,
                     "s_ptr": 0}
            if NQ > 2:
                state["fillers"] = project_fillers(2)
                state["fillers_w"] = 2
            POPS = {0: 8, 1: 6, 2: 5, 3: 6}
            pss = {}
            pos = {}

            def pop_filler():
                if state["fillers"]:
                    state["fillers"].pop(0)()
                    if not state["fillers"]:
                        state["proj_emitted"] = max(
                            state["proj_emitted"], state["fillers_w"])
                else:
                    dummy_mm()

            def drain_fillers():
                while state["fillers"]:
                    state["fillers"].pop(0)()
                state["proj_emitted"] = max(
                    state["proj_emitted"], state["fillers_w"])

            def emit_S_upto(limit):
                while state["s_ptr"] < min(limit, nsteps):
                    si, sk = steps[state["s_ptr"]]
                    if si > state["proj_emitted"]:
                        break
                    for j in (2 * sk, 2 * sk + 1):
                        pss[(si, j)] = s_mm(si, j)
                    state["s_ptr"] += 1

            cur_block = 0
            emit_S_upto(2)
            for s, (i, k) in enumerate(steps):
                if i != cur_block:
                    drain_fillers()
                    cur_block = i
                    if i + 2 < NQ:
                        state["fillers"] = project_fillers(i + 2)
                        state["fillers_w"] = i + 2
                    emit_S_upto(s + 2)
                nj = 4 * (i + 1)
                if k == 0:
                    pos[i] = accp.tile([P, QT], f32, tag="po", name=f"po{i}")
                po = pos[i]
                pts = {}
                for j in (2 * k, 2 * k + 1):
                    pts[j] = exp_mask(i, j, pss.pop((i, j)))
                emit_S_upto(s + 3)
                for j in (2 * k, 2 * k + 1):
                    w = width(i, j)
                    pt = pts.pop(j)
                    # PV (M=64, strips 0-1) + row sums (M=1, strip 2,
                    # partition 64 of the same bank) run concurrently
                    nc.tensor.matmul(
                        po[0:H, QT - w : QT],
                        vsb[:, j * H : (j + 1) * H],
                        pt[:, 0:w],
                        start=(j == 0), stop=(j == nj - 1))
                    nc.tensor.matmul(
                        po[H : H + 1, QT - w : QT],
                        ones,
                        pt[:, 0:w],
                        start=(j == 0), stop=(j == nj - 1))
                for _ in range(POPS.get(i, 3)):
                    pop_filler()
                if k == 2 * (i + 1) - 1:
                    finish_block(i, pos.pop(i))

    nc.compile()
    return nc


_NC_CACHE = None


def _get_nc():
    global _NC_CACHE
    if _NC_CACHE is None:
        _NC_CACHE = build_nc()
    return _NC_CACHE


def run(in_maps, trace=False, **kw):
    nc = _get_nc()
    return run_bass_kernel_spmd(nc, in_maps, core_ids=list(range(B)),
                                trace=trace, **kw)


def _pack_weights(Wq, Wk, Wv):
    """Host-side layout packing (pure permutation + constant folding).

    First tensor: [Wk | Wv] per chunk (M=128 kv projection).
    Second tensor: Wq * (1/sqrt(H)) per chunk (M=64 q projection)."""
    wkv = np.empty((P, NCH * P), dtype=np.float32)
    wq = np.empty((P, NCH * H), dtype=np.float32)
    scale = np.float32(1.0 / np.sqrt(H))
    for c in range(NCH):
        rows = slice(c * P, (c + 1) * P)
        wkv[:, c * P : c * P + H] = Wk[rows, :]
        wkv[:, c * P + H : (c + 1) * P] = Wv[rows, :]
        wq[:, c * H : (c + 1) * H] = Wq[rows, :] * scale
    return wkv, wq


def make_in_maps(x, Wq, Wk, Wv):
    x = np.asarray(x, dtype=np.float32)
    Wq = np.asarray(Wq, dtype=np.float32)
    Wk = np.asarray(Wk, dtype=np.float32)
    Wv = np.asarray(Wv, dtype=np.float32)
    wkq, wv = _pack_weights(Wq, Wk, Wv)
    ident = np.eye(P, dtype=BF16NP)
    return [
        {"xT": np.ascontiguousarray(x[b].T), "Wkq": wkq, "Wvp": wv,
         "IdD": ident}
        for b in range(B)
    ]


def kernel(x, Wq, Wk, Wv):
    res = run(make_in_maps(x, Wq, Wk, Wv))
    return np.stack([res.results[b]["out"] for b in range(B)], axis=0)


# revision 21
# speedup vs baseline: 1.2042x; 1.0326x over previous
"""Single-head causal attention (B=8, T=2048, C=1024, H=64) on 8 NeuronCores.

Data-parallel over batch: core b computes attention for x[b].

v8 design notes (v1 122.6us ... v6 72.2us):
  * Host stages x transposed; weights host-packed; 1/sqrt(H) folded
    into Wq; bf16 identity shipped from host.
  * Projections are three M=64 col-tiled streams per chunk (k -> PSUM
    partitions 0-63, q -> 64-127 of the same bank, v -> a second bank):
    adjacent matmuls on disjoint column strips run concurrently in the
    PE array, and q lands directly in the partition-64 copy the odd
    score matmuls need (kT-hi / qT-lo copies via SBUF->SBUF DMA).
  * PV drops the ones-column (M=64) and the softmax row sums come from
    a concurrent M=1 col-tiled matmul (stationary ones vector) into
    partition 64 of the same accumulator bank, so a PV step costs one
    N=512 stream instead of a serial M=65 matmul.
  * Attention is one global software pipeline: S-pairs (row-packed 2x
    via partition-64 operand copies) run ahead, exp+mask chase, PV
    chases; projection/v-transpose fillers and low-priority dummy
    matmuls keep the PE dense so HAM stays at K=8/8.
  * Diagonal tiles use reduced query width for S/exp/mask/PV/sums.
"""

import numpy as np
import ml_dtypes

import concourse.bass as bass
import concourse.bacc as bacc
import concourse.mybir as mybir
import concourse.tile as tile
from concourse.bass_utils import run_bass_kernel_spmd

B = 8
T, C, H = 2048, 1024, 64
P = 128
NCH = C // P     # 8 C-chunks
NT = T // P      # 16 T-tiles
QT = 512         # query-block width
NQ = T // QT     # 4 query blocks
H1 = H + 1
f32 = mybir.dt.float32
bf16 = mybir.dt.bfloat16
EXP = mybir.ActivationFunctionType.Exp
BF16NP = np.dtype(ml_dtypes.bfloat16)


def build_nc() -> bass.Bass:
    nc = bacc.Bacc("TRN2", target_bir_lowering=False, debug=False)
    xT = nc.dram_tensor("xT", [C, T], f32, kind="ExternalInput")
    Wkq = nc.dram_tensor("Wkq", [P, NCH * P], f32, kind="ExternalInput")
    Wvp = nc.dram_tensor("Wvp", [P, NCH * H], f32, kind="ExternalInput")
    IdD = nc.dram_tensor("IdD", [P, P], bf16, kind="ExternalInput")
    out = nc.dram_tensor("out", [T, H], f32, kind="ExternalOutput")

    with tile.TileContext(nc) as tc:
        with (
            tc.tile_pool(name="const", bufs=1) as constp,
            tc.tile_pool(name="w", bufs=1) as wp,
            tc.tile_pool(name="xt", bufs=3) as xtp,
            tc.tile_pool(name="qkv", bufs=1) as qkvp,
            tc.tile_pool(name="pt", bufs=6) as ptp,
            tc.tile_pool(name="fin", bufs=4) as finp,
            tc.tile_pool(name="ps", bufs=2, space="PSUM") as psp,    # kv/q chains
            tc.tile_pool(name="sps", bufs=4, space="PSUM") as spsp,  # S/pv/pob
            tc.tile_pool(name="acc", bufs=1, space="PSUM") as accp,  # po
            tc.tile_pool(name="junk", bufs=1, space="PSUM") as junkp,
        ):
            # identity from host via sync HWDGE (nothing queued ahead of it)
            ident = constp.tile([P, P], bf16, tag="ident")
            nc.sync.dma_start(out=ident, in_=IdD[:, :])

            # --- gpsimd SWDGE queue: weights, then per-chunk x windows ---
            wkq_r = wp.tile([P, NCH * P], bf16, tag="wkq_r")
            wv_r = wp.tile([P, NCH * H], bf16, tag="wv_r")
            nc.gpsimd.dma_start(out=wkq_r, in_=Wkq[:, :])
            nc.gpsimd.dma_start(out=wv_r, in_=Wvp[:, :])
            xvs = []
            for w in range(NQ):
                xtw = xtp.tile([P, NCH * QT], bf16, tag="xtw", name=f"xtw{w}")
                xv = xtw.rearrange("p (c t) -> p c t", t=QT)
                for c in range(NCH):
                    nc.gpsimd.dma_start(
                        out=xv[:, c, :],
                        in_=xT[c * P : (c + 1) * P, w * QT : (w + 1) * QT])
                xvs.append(xv)

            # --- persistent SBUF tensors ---
            kq = qkvp.tile([P, 2 * T], bf16, tag="kq")   # [0:T]=kT, [T:2T]=qT
            vt = qkvp.tile([P, T], bf16, tag="vt")       # vT at partitions 64-127
            vsb = qkvp.tile([P, NT * H], bf16, tag="vsb")  # v natural
            vsb_v = vsb.rearrange("p (t w) -> p t w", w=H)
            ones = constp.tile([P, 1], bf16, tag="ones")
            nc.vector.memset(ones, 1.0)
            osb = finp.tile([P, NT * H], f32, tag="osb", bufs=1)

            # --- reusable dummy-matmul filler (keeps HAM busy), low prio ---
            jt = junkp.tile([P, P], f32, tag="junk")

            def dummy_mm():
                nc.tensor.matmul(jt, ident, ident, start=True, stop=True)

            for _ in range(36):
                dummy_mm()

            def project_fillers(w):
                """PE-op closures for projections + v-transpose of window w.

                kv packed (M=128) + q (M=64) per chunk, interleaved so the
                chain advances at DMA chunk-arrival pace."""
                xv = xvs[w]
                kvp = psp.tile([P, QT], f32, tag="big", name=f"kv{w}")
                qp = psp.tile([P, QT], f32, tag="big", name=f"q{w}")
                ops = []
                for c in range(NCH):
                    ops.append(lambda c=c: nc.tensor.matmul(
                        kvp, wkq_r[:, c * P : (c + 1) * P], xv[:, c, :],
                        start=(c == 0), stop=(c == NCH - 1)))
                    ops.append(lambda c=c: nc.tensor.matmul(
                        qp[0:H, :], wv_r[:, c * H : (c + 1) * H], xv[:, c, :],
                        start=(c == 0), stop=(c == NCH - 1)))

                def casts():
                    cols = slice(w * QT, (w + 1) * QT)
                    qcols = slice(T + w * QT, T + (w + 1) * QT)
                    nc.vector.tensor_copy(kq[0:H, cols], kvp[0:H, :])
                    nc.vector.tensor_copy(kq[0:H, qcols], qp[0:H, :])
                    nc.vector.tensor_copy(vt[H:P, cols], kvp[H:P, :])
                    kq_pair = kq.rearrange("p (s t) -> p s t", s=2)
                    nc.sync.dma_start(
                        out=kq_pair[H:P, :, w * QT : (w + 1) * QT],
                        in_=kq_pair[0:H, :, w * QT : (w + 1) * QT])
                ops.append(casts)

                pv = spsp.tile([P, 4 * H], f32, tag="sps", name=f"pv{w}")
                for k in range(4):
                    ops.append(lambda k=k: nc.tensor.matmul(
                        pv[:, k * H : (k + 1) * H],
                        vt[H:P, (4 * w + k) * P : (4 * w + k + 1) * P],
                        ident[H:P, H:P], start=True, stop=True))
                ops.append(lambda: nc.vector.tensor_copy(
                    vsb_v[:, 4 * w : 4 * w + 4, :],
                    pv.rearrange("p (t u) -> p t u", u=H)))
                return ops

            # ---- global attention pipeline across blocks ----
            def width(i, j):
                d = j - 4 * i
                return QT - d * P if d > 0 else QT

            def s_mm(i, j):
                w = width(i, j)
                ps = spsp.tile([P, QT], f32, tag="sps", name=f"s{i}_{j}")
                rows = slice(0, H) if j % 2 == 0 else slice(H, P)
                qoff = T + i * QT + (QT - w)
                nc.tensor.matmul(
                    ps[:, 0:w],
                    kq[rows, j * P : (j + 1) * P],
                    kq[rows, qoff : qoff + w],
                    start=True, stop=True)
                return ps

            def exp_mask(i, j, ps):
                w = width(i, j)
                pt = ptp.tile([P, QT], bf16, tag="pt", name=f"pt{i}_{j}")
                nc.scalar.activation(pt[:, 0:w], ps[:, 0:w], EXP)
                if j >= 4 * i:
                    nc.gpsimd.affine_select(
                        out=pt[:, 0:w], in_=pt[:, 0:w],
                        pattern=[[1, w]],
                        compare_op=mybir.AluOpType.is_ge, fill=0.0,
                        base=0, channel_multiplier=-1)
                return pt

            def finish_block(i, po):
                ot = finp.tile([H1, QT], bf16, tag="ot")
                nc.vector.tensor_copy(ot, po[0:H1, :])
                pob = spsp.tile([P, 4 * H1], f32, tag="sps", name=f"pob{i}")
                for b in range(4):
                    nc.tensor.matmul(
                        pob[:, b * H1 : (b + 1) * H1],
                        ot[:, b * P : (b + 1) * P],
                        ident[0:H1, 0:H1], start=True, stop=True)
                for b in range(4):
                    t = 4 * i + b
                    rcp = finp.tile([P, 1], f32, tag="rcp")
                    nc.vector.reciprocal(rcp, pob[:, b * H1 + H : b * H1 + H1])
                    nc.vector.tensor_scalar_mul(
                        osb[:, t * H : (t + 1) * H],
                        pob[:, b * H1 : b * H1 + H], rcp)
                nc.sync.dma_start(
                    out=out.rearrange("(t p) h -> p t h", p=P)[:, 4 * i : 4 * i + 4, :],
                    in_=osb.rearrange("p (t h) -> p t h", h=H)[:, 4 * i : 4 * i + 4, :])

            steps = [(i, k) for i in range(NQ) for k in range(2 * (i + 1))]
            nsteps = len(steps)

            # windows 0 and 1 projected up front (DMA-paced anyway);
            # window w+2 projected as fillers inside attention block w
            for op in project_fillers(0):
                op()
            if NQ > 1:
                for op in project_fillers(1):
                    op()

            state = {"fillers": [], "fillers_w": 1, "proj_emitted": 1,
                     "s_ptr": 0}
            if NQ > 2:
                state["fillers"] = project_fillers(2)
                state["fillers_w"] = 2
            POPS = {0: 8, 1: 6, 2: 5, 3: 6}
            pss = {}
            pos = {}

            def pop_filler():
                if state["fillers"]:
                    state["fillers"].pop(0)()
                    if not state["fillers"]:
                        state["proj_emitted"] = max(
                            state["proj_emitted"], state["fillers_w"])
                else:
                    dummy_mm()

            def drain_fillers():
                while state["fillers"]:
                    state["fillers"].pop(0)()
                state["proj_emitted"] = max(
                    state["proj_emitted"], state["fillers_w"])

            def emit_S_upto(limit):
                while state["s_ptr"] < min(limit, nsteps):
                    si, sk = steps[state["s_ptr"]]
                    if si > state["proj_emitted"]:
                        break
                    for j in (2 * sk, 2 * sk + 1):
                        pss[(si, j)] = s_mm(si, j)
                    state["s_ptr"] += 1

            cur_block = 0
            emit_S_upto(2)
            for s, (i, k) in enumerate(steps):
                if i != cur_block:
                    drain_fillers()
                    cur_block = i
                    if i + 2 < NQ:
                        state["fillers"] = project_fillers(i + 2)
                        state["fillers_w"] = i + 2
                    emit_S_upto(s + 2)
                nj = 4 * (i + 1)
                if k == 0:
                    pos[i] = accp.tile([P, QT], f32, tag="po", name=f"po{i}")
                po = pos[i]
                pts = {}
                for j in (2 * k, 2 * k + 1):
                    pts[j] = exp_mask(i, j, pss.pop((i, j)))
                emit_S_upto(s + 3)
                for j in (2 * k, 2 * k + 1):
                    w = width(i, j)
                    pt = pts.pop(j)
                    # PV (M=64, strips 0-1) + row sums (M=1, strip 2,
                    # partition 64 of the same bank) run concurrently
                    nc.tensor.matmul(
                        po[0:H, QT - w : QT],
                        vsb[:, j * H : (j + 1) * H],
                        pt[:, 0:w],
                        start=(j == 0), stop=(j == nj - 1))
                    nc.tensor.matmul(
                        po[H : H + 1, QT - w : QT],
                        ones,
                        pt[:, 0:w],
                        start=(j == 0), stop=(j == nj - 1))
                for _ in range(POPS.get(i, 3)):
                    pop_filler()
                if k == 2 * (i + 1) - 1:
                    finish_block(i, pos.pop(i))

    nc.compile()
    return nc


_NC_CACHE = None


def _get_nc():
    global _NC_CACHE
    if _NC_CACHE is None:
        _NC_CACHE = build_nc()
    return _NC_CACHE


def run(in_maps, trace=False, **kw):
    nc = _get_nc()
    return run_bass_kernel_spmd(nc, in_maps, core_ids=list(range(B)),
                                trace=trace, **kw)


def _pack_weights(Wq, Wk, Wv):
    """Host-side layout packing (pure permutation + constant folding).

    First tensor: [Wk | Wv] per chunk (M=128 kv projection).
    Second tensor: Wq * (1/sqrt(H)) per chunk (M=64 q projection)."""
    wkv = np.empty((P, NCH * P), dtype=np.float32)
    wq = np.empty((P, NCH * H), dtype=np.float32)
    scale = np.float32(1.0 / np.sqrt(H))
    for c in range(NCH):
        rows = slice(c * P, (c + 1) * P)
        wkv[:, c * P : c * P + H] = Wk[rows, :]
        wkv[:, c * P + H : (c + 1) * P] = Wv[rows, :]
        wq[:, c * H : (c + 1) * H] = Wq[rows, :] * scale
    return wkv, wq


def make_in_maps(x, Wq, Wk, Wv):
    x = np.asarray(x, dtype=np.float32)
    Wq = np.asarray(Wq, dtype=np.float32)
    Wk = np.asarray(Wk, dtype=np.float32)
    Wv = np.asarray(Wv, dtype=np.float32)
    wkq, wv = _pack_weights(Wq, Wk, Wv)
    ident = np.eye(P, dtype=BF16NP)
    return [
        {"xT": np.ascontiguousarray(x[b].T), "Wkq": wkq, "Wvp": wv,
         "IdD": ident}
        for b in range(B)
    ]


def kernel(x, Wq, Wk, Wv):
    res = run(make_in_maps(x, Wq, Wk, Wv))
    return np.stack([res.results[b]["out"] for b in range(B)], axis=0)


# revision 22
# speedup vs baseline: 1.3325x; 1.1066x over previous
"""Single-head causal attention (B=8, T=2048, C=1024, H=64) on 8 NeuronCores.

Data-parallel over batch: core b computes attention for x[b].

v8 design notes (v1 122.6us ... v6 72.2us):
  * Host stages x transposed; weights host-packed; 1/sqrt(H) folded
    into Wq; bf16 identity shipped from host.
  * Projections are three M=64 col-tiled streams per chunk (k -> PSUM
    partitions 0-63, q -> 64-127 of the same bank, v -> a second bank):
    adjacent matmuls on disjoint column strips run concurrently in the
    PE array, and q lands directly in the partition-64 copy the odd
    score matmuls need (kT-hi / qT-lo copies via SBUF->SBUF DMA).
  * PV drops the ones-column (M=64) and the softmax row sums come from
    a concurrent M=1 col-tiled matmul (stationary ones vector) into
    partition 64 of the same accumulator bank, so a PV step costs one
    N=512 stream instead of a serial M=65 matmul.
  * Attention is one global software pipeline: S-pairs (row-packed 2x
    via partition-64 operand copies) run ahead, exp+mask chase, PV
    chases; projection/v-transpose fillers and low-priority dummy
    matmuls keep the PE dense so HAM stays at K=8/8.
  * Diagonal tiles use reduced query width for S/exp/mask/PV/sums.
"""

import numpy as np
import ml_dtypes

import concourse.bass as bass
import concourse.bacc as bacc
import concourse.mybir as mybir
import concourse.tile as tile
from concourse.bass_utils import run_bass_kernel_spmd

B = 8
T, C, H = 2048, 1024, 64
P = 128
NCH = C // P     # 8 C-chunks
NT = T // P      # 16 T-tiles
QT = 512         # query-block width
NQ = T // QT     # 4 query blocks
H1 = H + 1
f32 = mybir.dt.float32
bf16 = mybir.dt.bfloat16
EXP = mybir.ActivationFunctionType.Exp
BF16NP = np.dtype(ml_dtypes.bfloat16)


def build_nc() -> bass.Bass:
    nc = bacc.Bacc("TRN2", target_bir_lowering=False, debug=False)
    xT = nc.dram_tensor("xT", [C, T], f32, kind="ExternalInput")
    Wkq = nc.dram_tensor("Wkq", [P, NCH * P], f32, kind="ExternalInput")
    Wvp = nc.dram_tensor("Wvp", [P, NCH * H], f32, kind="ExternalInput")
    IdD = nc.dram_tensor("IdD", [P, P], bf16, kind="ExternalInput")
    out = nc.dram_tensor("out", [T, H], f32, kind="ExternalOutput")

    with tile.TileContext(nc) as tc:
        with (
            tc.tile_pool(name="const", bufs=1) as constp,
            tc.tile_pool(name="w", bufs=1) as wp,
            tc.tile_pool(name="xt", bufs=3) as xtp,
            tc.tile_pool(name="qkv", bufs=1) as qkvp,
            tc.tile_pool(name="pt", bufs=6) as ptp,
            tc.tile_pool(name="fin", bufs=4) as finp,
            tc.tile_pool(name="ps", bufs=2, space="PSUM") as psp,    # kv/q chains
            tc.tile_pool(name="sps", bufs=4, space="PSUM") as spsp,  # S/pv/pob
            tc.tile_pool(name="acc", bufs=1, space="PSUM") as accp,  # po
            tc.tile_pool(name="junk", bufs=1, space="PSUM") as junkp,
        ):
            # identity from host via sync HWDGE (nothing queued ahead of it)
            ident = constp.tile([P, P], bf16, tag="ident")
            nc.sync.dma_start(out=ident, in_=IdD[:, :])

            # --- gpsimd SWDGE queue: weights, then per-chunk x windows ---
            wkq_r = wp.tile([P, NCH * P], bf16, tag="wkq_r")
            wv_r = wp.tile([P, NCH * H], bf16, tag="wv_r")
            nc.gpsimd.dma_start(out=wkq_r, in_=Wkq[:, :])
            nc.gpsimd.dma_start(out=wv_r, in_=Wvp[:, :])
            xvs = []
            for w in range(NQ):
                xtw = xtp.tile([P, NCH * QT], bf16, tag="xtw", name=f"xtw{w}")
                xv = xtw.rearrange("p (c t) -> p c t", t=QT)
                for c in range(NCH):
                    nc.gpsimd.dma_start(
                        out=xv[:, c, :],
                        in_=xT[c * P : (c + 1) * P, w * QT : (w + 1) * QT])
                xvs.append(xv)

            # --- persistent SBUF tensors ---
            kq = qkvp.tile([P, 2 * T], bf16, tag="kq")   # [0:T]=kT, [T:2T]=qT
            vt = qkvp.tile([P, T], bf16, tag="vt")       # vT at partitions 64-127
            vsb = qkvp.tile([P, NT * H1], bf16, tag="vsb")  # v natural + ones
            vsb_v = vsb.rearrange("p (t w) -> p t w", w=H1)
            ones = constp.tile([P, NT], f32, tag="ones")
            nc.vector.memset(ones, 1.0)
            nc.vector.tensor_copy(vsb_v[:, :, H:H1], ones.unsqueeze(2))
            osb = finp.tile([P, NT * H], f32, tag="osb", bufs=1)

            # --- reusable dummy-matmul filler (keeps HAM busy), low prio ---
            jt = junkp.tile([P, P], f32, tag="junk")

            def dummy_mm():
                nc.tensor.matmul(jt, ident, ident, start=True, stop=True)

            for _ in range(36):
                dummy_mm()

            def project_fillers(w):
                """PE-op closures for projections + v-transpose of window w.

                kv packed (M=128) + q (M=64) per chunk, interleaved so the
                chain advances at DMA chunk-arrival pace."""
                xv = xvs[w]
                kvp = psp.tile([P, QT], f32, tag="big", name=f"kv{w}")
                qp = psp.tile([P, QT], f32, tag="big", name=f"q{w}")
                ops = []
                for c in range(NCH):
                    ops.append(lambda c=c: nc.tensor.matmul(
                        kvp, wkq_r[:, c * P : (c + 1) * P], xv[:, c, :],
                        start=(c == 0), stop=(c == NCH - 1)))
                    ops.append(lambda c=c: nc.tensor.matmul(
                        qp[0:H, :], wv_r[:, c * H : (c + 1) * H], xv[:, c, :],
                        start=(c == 0), stop=(c == NCH - 1)))

                def casts():
                    cols = slice(w * QT, (w + 1) * QT)
                    qcols = slice(T + w * QT, T + (w + 1) * QT)
                    nc.vector.tensor_copy(kq[0:H, cols], kvp[0:H, :])
                    nc.vector.tensor_copy(kq[0:H, qcols], qp[0:H, :])
                    nc.vector.tensor_copy(vt[H:P, cols], kvp[H:P, :])
                    kq_pair = kq.rearrange("p (s t) -> p s t", s=2)
                    nc.sync.dma_start(
                        out=kq_pair[H:P, :, w * QT : (w + 1) * QT],
                        in_=kq_pair[0:H, :, w * QT : (w + 1) * QT])
                ops.append(casts)

                pv = spsp.tile([P, 4 * H], f32, tag="sps", name=f"pv{w}")
                for k in range(4):
                    ops.append(lambda k=k: nc.tensor.matmul(
                        pv[:, k * H : (k + 1) * H],
                        vt[H:P, (4 * w + k) * P : (4 * w + k + 1) * P],
                        ident[H:P, H:P], start=True, stop=True))
                ops.append(lambda: nc.vector.tensor_copy(
                    vsb_v[:, 4 * w : 4 * w + 4, 0:H],
                    pv.rearrange("p (t u) -> p t u", u=H)))
                return ops

            # ---- global attention pipeline across blocks ----
            def width(i, j):
                d = j - 4 * i
                return QT - d * P if d > 0 else QT

            def s_mm(i, j):
                w = width(i, j)
                ps = spsp.tile([P, QT], f32, tag="sps", name=f"s{i}_{j}")
                rows = slice(0, H) if j % 2 == 0 else slice(H, P)
                qoff = T + i * QT + (QT - w)
                nc.tensor.matmul(
                    ps[:, 0:w],
                    kq[rows, j * P : (j + 1) * P],
                    kq[rows, qoff : qoff + w],
                    start=True, stop=True)
                return ps

            def exp_mask(i, j, ps):
                w = width(i, j)
                pt = ptp.tile([P, QT], bf16, tag="pt", name=f"pt{i}_{j}")
                nc.scalar.activation(pt[:, 0:w], ps[:, 0:w], EXP)
                if j >= 4 * i:
                    nc.gpsimd.affine_select(
                        out=pt[:, 0:w], in_=pt[:, 0:w],
                        pattern=[[1, w]],
                        compare_op=mybir.AluOpType.is_ge, fill=0.0,
                        base=0, channel_multiplier=-1)
                return pt

            def finish_block(i, po):
                ot = finp.tile([H1, QT], bf16, tag="ot")
                nc.vector.tensor_copy(ot, po[0:H1, :])
                pob = spsp.tile([P, 4 * H1], f32, tag="sps", name=f"pob{i}")
                for b in range(4):
                    nc.tensor.matmul(
                        pob[:, b * H1 : (b + 1) * H1],
                        ot[:, b * P : (b + 1) * P],
                        ident[0:H1, 0:H1], start=True, stop=True)
                for b in range(4):
                    t = 4 * i + b
                    rcp = finp.tile([P, 1], f32, tag="rcp")
                    nc.vector.reciprocal(rcp, pob[:, b * H1 + H : b * H1 + H1])
                    nc.vector.tensor_scalar_mul(
                        osb[:, t * H : (t + 1) * H],
                        pob[:, b * H1 : b * H1 + H], rcp)
                nc.sync.dma_start(
                    out=out.rearrange("(t p) h -> p t h", p=P)[:, 4 * i : 4 * i + 4, :],
                    in_=osb.rearrange("p (t h) -> p t h", h=H)[:, 4 * i : 4 * i + 4, :])

            steps = [(i, k) for i in range(NQ) for k in range(2 * (i + 1))]
            nsteps = len(steps)

            # windows 0 and 1 projected up front (DMA-paced anyway);
            # window w+2 projected as fillers inside attention block w
            for op in project_fillers(0):
                op()
            if NQ > 1:
                for op in project_fillers(1):
                    op()

            state = {"fillers": [], "fillers_w": 1, "proj_emitted": 1,
                     "s_ptr": 0}
            if NQ > 2:
                state["fillers"] = project_fillers(2)
                state["fillers_w"] = 2
            POPS = {0: 8, 1: 6, 2: 5, 3: 6}
            pss = {}
            pos = {}

            def pop_filler():
                if state["fillers"]:
                    state["fillers"].pop(0)()
                    if not state["fillers"]:
                        state["proj_emitted"] = max(
                            state["proj_emitted"], state["fillers_w"])
                else:
                    dummy_mm()

            def drain_fillers():
                while state["fillers"]:
                    state["fillers"].pop(0)()
                state["proj_emitted"] = max(
                    state["proj_emitted"], state["fillers_w"])

            def emit_S_upto(limit):
                while state["s_ptr"] < min(limit, nsteps):
                    si, sk = steps[state["s_ptr"]]
                    if si > state["proj_emitted"]:
                        break
                    for j in (2 * sk, 2 * sk + 1):
                        pss[(si, j)] = s_mm(si, j)
                    state["s_ptr"] += 1

            cur_block = 0
            emit_S_upto(2)
            for s, (i, k) in enumerate(steps):
                if i != cur_block:
                    drain_fillers()
                    cur_block = i
                    if i + 2 < NQ:
                        state["fillers"] = project_fillers(i + 2)
                        state["fillers_w"] = i + 2
                    emit_S_upto(s + 2)
                nj = 4 * (i + 1)
                if k == 0:
                    pos[i] = accp.tile([P, QT], f32, tag="po", name=f"po{i}")
                po = pos[i]
                pts = {}
                for j in (2 * k, 2 * k + 1):
                    pts[j] = exp_mask(i, j, pss.pop((i, j)))
                emit_S_upto(s + 3)
                for j in (2 * k, 2 * k + 1):
                    w = width(i, j)
                    nc.tensor.matmul(
                        po[0:H1, QT - w : QT],
                        vsb[:, j * H1 : (j + 1) * H1],
                        pts.pop(j)[:, 0:w],
                        start=(j == 0), stop=(j == nj - 1))
                for _ in range(POPS.get(i, 3)):
                    pop_filler()
                if k == 2 * (i + 1) - 1:
                    finish_block(i, pos.pop(i))

    nc.compile()
    return nc


_NC_CACHE = None


def _get_nc():
    global _NC_CACHE
    if _NC_CACHE is None:
        _NC_CACHE = build_nc()
    return _NC_CACHE


def run(in_maps, trace=False, **kw):
    nc = _get_nc()
    return run_bass_kernel_spmd(nc, in_maps, core_ids=list(range(B)),
                                trace=trace, **kw)


def _pack_weights(Wq, Wk, Wv):
    """Host-side layout packing (pure permutation + constant folding).

    First tensor: [Wk | Wv] per chunk (M=128 kv projection).
    Second tensor: Wq * (1/sqrt(H)) per chunk (M=64 q projection)."""
    wkv = np.empty((P, NCH * P), dtype=np.float32)
    wq = np.empty((P, NCH * H), dtype=np.float32)
    scale = np.float32(1.0 / np.sqrt(H))
    for c in range(NCH):
        rows = slice(c * P, (c + 1) * P)
        wkv[:, c * P : c * P + H] = Wk[rows, :]
        wkv[:, c * P + H : (c + 1) * P] = Wv[rows, :]
        wq[:, c * H : (c + 1) * H] = Wq[rows, :] * scale
    return wkv, wq


def make_in_maps(x, Wq, Wk, Wv):
    x = np.asarray(x, dtype=np.float32)
    Wq = np.asarray(Wq, dtype=np.float32)
    Wk = np.asarray(Wk, dtype=np.float32)
    Wv = np.asarray(Wv, dtype=np.float32)
    wkq, wv = _pack_weights(Wq, Wk, Wv)
    ident = np.eye(P, dtype=BF16NP)
    return [
        {"xT": np.ascontiguousarray(x[b].T), "Wkq": wkq, "Wvp": wv,
         "IdD": ident}
        for b in range(B)
    ]


def kernel(x, Wq, Wk, Wv):
    res = run(make_in_maps(x, Wq, Wk, Wv))
    return np.stack([res.results[b]["out"] for b in range(B)], axis=0)
